# revision 5
# baseline (speedup 1.0000x reference)
"""DSFourierAttention Trainium2 kernel (fp8 redesign).

Math (per (b, h) slice, validated vs the jax reference):
    qf = rfft(q, ortho) etc. as dense DFT matmuls (Fre/Fim [L, X], X = L//2+1)
    qk_T[y, x] = sum_e (kfr qfr + kfi qfi)       (stacked [re; im] K=128 matmul)
    im_T[y, x] = sum_e (-kfi qfr + kfr qfi)      (kswp = [-kfi; kfr])
    p = exp(|qk|)/4                              (softmax-ratio invariant)
    qkv_T[x, e] = (p^T @ [vfr | vfi | ones]) / colsum
    out[l, e] = Gre^T @ qkvr + Gim^T @ qkvi      (irfft weights w = [1, 2.., 2, 1])
    out = out * tau[b] + delta[b, l]

Precision plan (numpy-simulated, rel err ~5.6e-3 vs 2e-2 budget):
    fp8 e4m3: q/k data+DFT weights (DoubleRow K=256 pairs), qstk/kstk/kswp,
    exp tiles, qkv, iFFT weights. bf16: whole v path (precision-critical).
Engine split for softmax: re^2 gpsimd, im^2 DVE, add gpsimd (bf16),
    sqrt+exp ACT (exp written fp8 in-place with bias=-ln4).
Sharding: batch-parallel, 2 batches per core across 8 cores.
"""

import os
import sys

import numpy as np

for _p in ("/opt/trn_rl_repo", "/root/.axon_site/_ro/trn_rl_repo"):
    if os.path.isdir(_p) and _p not in sys.path:
        sys.path.insert(0, _p)

import ml_dtypes  # noqa: E402
import concourse.bass as bass  # noqa: E402
import concourse.tile as tile  # noqa: E402
from concourse import bacc, mybir  # noqa: E402
from concourse.bass_utils import run_bass_kernel_spmd  # noqa: E402

B, L, H, E = 16, 1024, 8, 64
X = L // 2 + 1          # 513 rfft bins
XP = X + 1              # 514 (pad: dst free >= 2 for the bin-512 matmuls)
XP8 = 528               # fp8 const pitch: DoubleRow k-tile step must be %16
NCORES = 8
BL = B // NCORES        # 2 batches per core
NLC = L // 128          # 8 l-chunks
NYC = 4                 # full 128-row y chunks (y=512 handled as ragged row)
NXC = 4
NWAVE = 2               # softmax waves per batch (4 heads each)

SF = 32.0               # fp8 DFT weight scale (fre8 = fre * 32)
SG = 16.0               # fp8 irfft weight scale
SQKV = 64.0             # qkv fp8 scale
LN4 = 1.3862943611198906

F32 = mybir.dt.float32
BF16 = mybir.dt.bfloat16
F8 = mybir.dt.float8e4
AF = mybir.ActivationFunctionType
DR = mybir.MatmulPerfMode.DoubleRow

NPF8 = ml_dtypes.float8_e4m3
NPBF = ml_dtypes.bfloat16

LAST_RESULT = None


def _consts():
    l = np.arange(L)
    xs = np.arange(X)
    ang = 2.0 * np.pi * np.outer(l, xs) / L          # [L, X]
    fre = np.zeros((L, XP), NPBF)
    fim = np.zeros((L, XP), NPBF)
    fre[:, :X] = (np.cos(ang) / np.sqrt(L)).astype(NPBF)
    fim[:, :X] = (-np.sin(ang) / np.sqrt(L)).astype(NPBF)
    fre8 = np.zeros((L, XP8), NPF8)
    fim8 = np.zeros((L, XP8), NPF8)
    fre8[:, :X] = (np.cos(ang) / np.sqrt(L) * SF).astype(NPF8)
    fim8[:, :X] = (-np.sin(ang) / np.sqrt(L) * SF).astype(NPF8)
    w = np.full(X, 2.0)
    w[0] = 1.0
    w[-1] = 1.0
    gre8 = (w[:, None] * np.cos(ang.T) / np.sqrt(L) * SG).astype(NPF8)   # [X, L]
    gim8 = (w[:, None] * -np.sin(ang.T) / np.sqrt(L) * SG).astype(NPF8)
    return fre, fim, fre8, fim8, gre8, gim8


def build_module(bl=BL, compile=True):
    from concourse.alu_op_type import AluOpType

    nc = bacc.Bacc("TRN2", target_bir_lowering=False, debug=False,
                   num_devices=NCORES)

    qd = nc.dram_tensor("qd", [bl, L, H, E], F8, kind="ExternalInput").ap()
    kd = nc.dram_tensor("kd", [bl, L, H, E], F8, kind="ExternalInput").ap()
    vd = nc.dram_tensor("vd", [bl, L, H, E], BF16, kind="ExternalInput").ap()
    taud = nc.dram_tensor("taud", [bl, 1], F32, kind="ExternalInput").ap()
    deltad = nc.dram_tensor("deltad", [bl, L], F32, kind="ExternalInput").ap()
    fred = nc.dram_tensor("fred", [L, XP], BF16, kind="ExternalInput").ap()
    fimd = nc.dram_tensor("fimd", [L, XP], BF16, kind="ExternalInput").ap()
    fre8d = nc.dram_tensor("fre8d", [L, XP8], F8, kind="ExternalInput").ap()
    fim8d = nc.dram_tensor("fim8d", [L, XP8], F8, kind="ExternalInput").ap()
    gre8d = nc.dram_tensor("gre8d", [X, L], F8, kind="ExternalInput").ap()
    gim8d = nc.dram_tensor("gim8d", [X, L], F8, kind="ExternalInput").ap()
    outd = nc.dram_tensor("outd", [bl, L, H, E], F32, kind="ExternalOutput").ap()

    with tile.TileContext(nc) as tc:
        _body(nc, tc, AluOpType, qd, kd, vd, taud, deltad, fred, fimd,
              fre8d, fim8d, gre8d, gim8d, outd, bl)
    if compile:
        nc.compile()
    return nc


def _body(nc, tc, OPS, qd, kd, vd, taud, deltad, fred, fimd,
          fre8d, fim8d, gre8d, gim8d, outd, bl=BL):
    from contextlib import ExitStack

    ctx = ExitStack()
    with ctx:
        consts = ctx.enter_context(tc.tile_pool(name="consts", bufs=1))
        io = ctx.enter_context(tc.tile_pool(name="io", bufs=2))
        stg = ctx.enter_context(tc.tile_pool(name="stg", bufs=2))
        stk = ctx.enter_context(tc.tile_pool(name="stk", bufs=4))
        wv = ctx.enter_context(tc.tile_pool(name="wv", bufs=17))
        sm = ctx.enter_context(tc.tile_pool(name="sm", bufs=3))
        vfp = ctx.enter_context(tc.tile_pool(name="vfp", bufs=5))
        qkvp = ctx.enter_context(tc.tile_pool(name="qkvp", bufs=2))
        ep = ctx.enter_context(tc.tile_pool(name="ep", bufs=2))
        pf = ctx.enter_context(tc.tile_pool(name="pf", bufs=3, space="PSUM"))
        ph = ctx.enter_context(tc.tile_pool(name="ph", bufs=2, space="PSUM"))

        # ---- bf16 DFT consts (v path) -- needed first ------------------
        fre_sb = consts.tile([128, NLC, XP], BF16)
        fim_sb = consts.tile([128, NLC, XP], BF16)
        for c in range(NLC):
            nc.sync.dma_start(
                out=fre_sb[:, c, :],
                in_=fred.rearrange("(c p) x -> p c x", p=128)[:, c, :])
            nc.sync.dma_start(
                out=fim_sb[:, c, :],
                in_=fimd.rearrange("(c p) x -> p c x", p=128)[:, c, :])

        # exp bias (-ln 4)
        ebias = consts.tile([128, 1], F32)
        nc.vector.memset(ebias[:, :], -LN4)

        vf_next = _phase_v(nc, 0, vd, fre_sb, fim_sb, io, vfp, pf, ph)

        # ---- fp8 consts (q/k fft, irfft) ------------------------------
        f8_re = consts.tile([128, NLC, XP8], F8)
        f8_im = consts.tile([128, NLC, XP8], F8)
        for c in range(NLC):
            nc.sync.dma_start(
                out=f8_re[:, c, :],
                in_=fre8d.rearrange("(c p) x -> p c x", p=128)[:, c, :])
            nc.sync.dma_start(
                out=f8_im[:, c, :],
                in_=fim8d.rearrange("(c p) x -> p c x", p=128)[:, c, :])
        g8_re = consts.tile([128, NXC, L], F8)
        g8_im = consts.tile([128, NXC, L], F8)
        for c in range(NXC):
            nc.sync.dma_start(
                out=g8_re[:, c, :],
                in_=gre8d[0:512].rearrange("(c p) l -> p c l", p=128)[:, c, :])
            nc.sync.dma_start(
                out=g8_im[:, c, :],
                in_=gim8d[0:512].rearrange("(c p) l -> p c l", p=128)[:, c, :])
        g512 = consts.tile([1, L], F8)
        nc.sync.dma_start(out=g512[0:1, :], in_=gre8d[512:513, :])

        for b in range(bl):
            vf_next = _batch(nc, tc, OPS, b, bl, qd, kd, vd, taud, deltad,
                             outd, fre_sb, fim_sb, f8_re, f8_im,
                             g8_re, g8_im, g512, ebias,
                             io, stg, stk, wv, sm, vfp, qkvp, ep, pf, ph,
                             vf_next)


def _phase_v(nc, b, vd, fre_sb, fim_sb, io, vfp, pf, ph):
    """Load v[b] (bf16) and compute the transposed FFT into vf_av/v512."""
    v_sb = io.tile([128, NLC, H * E], BF16, tag="vsb", bufs=2,
                   name=f"vsb{b}")
    for c in range(NLC):
        nc.sync.dma_start(
            out=v_sb[:, c, :],
            in_=vd[b].rearrange("(c p) h e -> p c (h e)", p=128)[:, c, :])

    vf_av = []
    for yc in range(NYC):
        t = vfp.tile([128, H, 132], BF16, tag="vfav", bufs=8,
                     name=f"vfav{b}_{yc}")
        vf_av.append(t)
    v512 = vfp.tile([1, H, 132], BF16, tag="v512", bufs=2, name=f"v512_{b}")

    for part, f_sb in ((0, fre_sb), (1, fim_sb)):
        for yc in range(NYC):
            ps = pf.tile([128, 520], F32, tag="pf", name=f"psv{b}_{part}_{yc}")
            for c in range(NLC):
                nc.tensor.matmul(ps[:, 0:512],
                                 f_sb[:, c, yc * 128:(yc + 1) * 128],
                                 v_sb[:, c, :],
                                 start=(c == 0), stop=(c == NLC - 1))
            nc.scalar.copy(
                out=vf_av[yc][:, :, part * 64:(part + 1) * 64],
                in_=ps[:, 0:512].rearrange("p (h e) -> p h e", h=H))
    for yc in range(NYC):
        nc.vector.memset(vf_av[yc][:, :, 128:129], 1.0)

    # ragged y=512 row of vf (imag is 0)
    ps512 = ph.tile([1, 512], F32, tag="ph", name=f"psv512_{b}")
    for c in range(NLC):
        nc.tensor.matmul(ps512[0:1, 0:512],
                         fre_sb[:, c, 512:513],
                         v_sb[:, c, :],
                         start=(c == 0), stop=(c == NLC - 1))
    nc.scalar.copy(out=v512[0:1, :, 0:64],
                   in_=ps512[0:1, 0:512].rearrange("p (h e) -> p h e", h=H))
    nc.vector.memset(v512[0:1, :, 64:128], 0.0)
    nc.vector.memset(v512[0:1, :, 128:129], 1.0)
    return vf_av, v512


def _batch(nc, tc, OPS, b, bl, qd, kd, vd, taud, deltad, outd,
           fre_sb, fim_sb, f8_re, f8_im, g8_re, g8_im, g512, ebias,
           io, stg, stk, wv, sm, vfp, qkvp, ep, pf, ph, vf_cur):
    vf_av, v512 = vf_cur

    # ---- epilogue scalars -------------------------------------------
    tau_sb = ep.tile([128, 1], F32, tag="tau")
    nc.sync.dma_start(out=tau_sb[:, :], in_=taud[b:b + 1, 0:1].to_broadcast([128, 1]))
    delta_sb = ep.tile([128, NLC], F32, tag="delta")
    nc.sync.dma_start(out=delta_sb[:, :],
                      in_=deltad[b, :].rearrange("(c p) -> p c", p=128))

    # ---- q/k all-heads fp8 loads ------------------------------------
    q8 = io.tile([128, NLC, H * E], F8, tag="q8", bufs=2, name=f"q8_{b}")
    k8 = io.tile([128, NLC, H * E], F8, tag="k8", bufs=2, name=f"k8_{b}")
    for c in range(NLC):
        nc.sync.dma_start(
            out=q8[:, c, :],
            in_=qd[b].rearrange("(c p) h e -> p c (h e)", p=128)[:, c, :])
        nc.sync.dma_start(
            out=k8[:, c, :],
            in_=kd[b].rearrange("(c p) h e -> p c (h e)", p=128)[:, c, :])

    # ---- qkv accumulator (fp8, one tile for DoubleRow pair APs) ------
    qkv8 = qkvp.tile([128, NXC, 2, H, 64], F8, tag="qkv", name=f"qkv{b}")
    qkv512 = qkvp.tile([1, 2, H, 64], F8, tag="qkv512", bufs=2,
                       name=f"qkv512_{b}")

    for w in range(NWAVE):
        _wave(nc, tc, OPS, b, w, q8, k8, f8_re, f8_im, ebias,
              io, stg, stk, wv, sm, pf, ph,
              vf_av, v512, qkv8, qkv512)

    # next batch's independent v-FFT emitted before the iFFT so the PE
    # has work while this batch's softmax/AV tail drains
    vf_next = None
    if b + 1 < bl:
        vf_next = _phase_v(nc, b + 1, vd, fre_sb, fim_sb, io, vfp, pf, ph)

    # ---- iFFT (fp8 DoubleRow) + epilogue, split by head halves ------
    for half in range(2):
        hs = slice(4 * half, 4 * half + 4)
        cs = slice(256 * half, 256 * half + 256)
        for lc in range(NLC):
            lcs = slice(lc * 128, (lc + 1) * 128)
            ps_o = ph.tile([128, 512], F32, tag="ph", name=f"pso{b}_{half}_{lc}")
            for j in range(2):
                nc.tensor.matmul(ps_o[:, 0:256],
                                 g8_re[:, 2 * j:2 * j + 2, lcs],
                                 qkv8[:, 2 * j:2 * j + 2, 0, hs, :],
                                 start=(j == 0), stop=False, perf_mode=DR)
                nc.tensor.matmul(ps_o[:, 0:256],
                                 g8_im[:, 2 * j:2 * j + 2, lcs],
                                 qkv8[:, 2 * j:2 * j + 2, 1, hs, :],
                                 start=False, stop=False, perf_mode=DR)
            nc.tensor.matmul(ps_o[:, 0:256],
                             g512[0:1, lcs],
                             qkv512[0:1, 0, hs, :],
                             start=False, stop=True)
            out_t = ep.tile([128, 256], F32, tag="outsb", bufs=4,
                            name=f"out{b}_{half}_{lc}")
            nc.vector.tensor_scalar(out=out_t[:, :], in0=ps_o[:, 0:256],
                                    scalar1=tau_sb[:, 0:1],
                                    scalar2=delta_sb[:, lc:lc + 1],
                                    op0=OPS.mult, op1=OPS.add)
            nc.sync.dma_start(
                out=outd[b, lcs, hs, :].rearrange("l h e -> l (h e)"),
                in_=out_t[:, :])
    return vf_next


def _wave(nc, tc, OPS, b, w, q8, k8, f8_re, f8_im, ebias,
          io, stg, stk, wv, sm, pf, ph,
          vf_av, v512, qkv8, qkv512):
    heads = [4 * w + i for i in range(4)]
    hps = [2 * w, 2 * w + 1]

    # ---- q/k FFT (fp8 DoubleRow over c-chunk pairs) -----------------
    qstk = {}
    kstk = {}
    kswp = {}
    for hp in hps:
        hcs = slice(hp * 128, (hp + 1) * 128)
        for t, src in ((0, q8), (1, k8)):
            ps_re = pf.tile([128, 520], F32, tag="pf", name=f"psfr{b}_{hp}_{t}")
            ps_im = pf.tile([128, 520], F32, tag="pf", name=f"psfi{b}_{hp}_{t}")
            for j in range(NLC // 2):
                lhsT = src[:, 2 * j:2 * j + 2, hcs]
                nc.tensor.matmul(ps_re[:, 0:512], lhsT,
                                 f8_re[:, 2 * j:2 * j + 2, 0:512],
                                 start=(j == 0), stop=(j == 3), perf_mode=DR)
                nc.tensor.matmul(ps_re[:, 512:516], lhsT,
                                 f8_re[:, 2 * j:2 * j + 2, 512:516],
                                 start=(j == 0), stop=(j == 3), perf_mode=DR)
                nc.tensor.matmul(ps_im[:, 0:512], lhsT,
                                 f8_im[:, 2 * j:2 * j + 2, 0:512],
                                 start=(j == 0), stop=(j == 3), perf_mode=DR)
                nc.tensor.matmul(ps_im[:, 512:516], lhsT,
                                 f8_im[:, 2 * j:2 * j + 2, 512:516],
                                 start=(j == 0), stop=(j == 3), perf_mode=DR)
            # staging casts: fold 1/SF (and q's 1/sqrt(E)) into the scale
            scale = (0.125 / SF) if t == 0 else (1.0 / SF)
            st_re = stg.tile([128, XP], F8, tag="stre", name=f"stre{b}_{hp}_{t}")
            nc.vector.tensor_scalar_mul(out=st_re[:, 0:514],
                                        in0=ps_re[:, 0:514], scalar1=scale)
            st_im = stg.tile([128, XP], F8, tag="stim", name=f"stim{b}_{hp}_{t}")
            nc.vector.tensor_scalar_mul(out=st_im[:, 0:514],
                                        in0=ps_im[:, 0:514], scalar1=scale)
            if t == 1:
                st_imn = stg.tile([128, XP], F8, tag="stimn",
                                  name=f"stimn{b}_{hp}")
                nc.vector.tensor_scalar_mul(out=st_imn[:, 0:514],
                                            in0=ps_im[:, 0:514],
                                            scalar1=-scale)

            for phi in range(2):
                h = 2 * hp + phi
                rows = slice(64 * phi, 64 * phi + 64)
                if t == 0:
                    dst = stk.tile([128, XP], F8, tag="qstk", name=f"qstk{b}_{h}")
                    qstk[h] = dst
                    nc.sync.dma_start(out=dst[0:64, :], in_=st_re[rows, :])
                    nc.sync.dma_start(out=dst[64:128, :], in_=st_im[rows, :])
                else:
                    dst = stk.tile([128, XP], F8, tag="kstk", name=f"kstk{b}_{h}")
                    kstk[h] = dst
                    nc.sync.dma_start(out=dst[0:64, :], in_=st_re[rows, :])
                    nc.sync.dma_start(out=dst[64:128, :], in_=st_im[rows, :])
                    dsw = stk.tile([128, XP], F8, tag="kswp", name=f"kswp{b}_{h}")
                    kswp[h] = dsw
                    nc.sync.dma_start(out=dsw[0:64, :], in_=st_imn[rows, :])
                    nc.sync.dma_start(out=dsw[64:128, :], in_=st_re[rows, :])

    # ---- QK (fp8) + magnitude + exp ---------------------------------
    u_tiles = {}
    rag_s = wv.tile([4, X], F32, tag="rags", bufs=2, name=f"rags{b}_{w}")
    rag_i = wv.tile([4, X], F32, tag="ragi", bufs=2, name=f"ragi{b}_{w}")
    for i, h in enumerate(heads):
        for yc in range(NYC):
            ps_r = pf.tile([128, 520], F32, tag="pf", name=f"psqr{b}_{h}_{yc}")
            ps_i = pf.tile([128, 520], F32, tag="pf", name=f"psqi{b}_{h}_{yc}")
            ycs = slice(yc * 128, (yc + 1) * 128)
            nc.tensor.matmul(ps_r[:, 0:512], kstk[h][:, ycs],
                             qstk[h][:, 0:512], start=True, stop=True)
            nc.tensor.matmul(ps_r[:, 512:514], kstk[h][:, ycs],
                             qstk[h][:, 512:514], start=True, stop=True)
            nc.tensor.matmul(ps_i[:, 0:512], kswp[h][:, ycs],
                             qstk[h][:, 0:512], start=True, stop=True)
            nc.tensor.matmul(ps_i[:, 512:514], kswp[h][:, ycs],
                             qstk[h][:, 512:514], start=True, stop=True)
            # |qk|^2 = re^2 + im^2: PSUM is only readable by DVE/ACT and
            # only one PSUM operand per op, so DVE drains both to bf16,
            # then gpsimd squares+adds in SBUF.
            u_t = wv.tile([128, XP], BF16, tag="ut", bufs=17,
                          name=f"ut{b}_{h}_{yc}")
            cr = sm.tile([128, XP], BF16, tag="cr", bufs=3,
                         name=f"cr{b}_{h}_{yc}")
            ci = sm.tile([128, XP], BF16, tag="ci", bufs=3,
                         name=f"ci{b}_{h}_{yc}")
            nc.vector.tensor_copy(out=cr[:, 0:513], in_=ps_r[:, 0:513])
            nc.vector.tensor_copy(out=ci[:, 0:513], in_=ps_i[:, 0:513])
            tmp = sm.tile([128, XP], BF16, tag="sqim", name=f"sqim{b}_{h}_{yc}")
            nc.gpsimd.tensor_mul(out=u_t[:, 0:513], in0=cr[:, 0:513],
                                 in1=cr[:, 0:513])
            nc.gpsimd.tensor_mul(out=tmp[:, 0:513], in0=ci[:, 0:513],
                                 in1=ci[:, 0:513])
            nc.gpsimd.tensor_add(out=u_t[:, 0:513], in0=u_t[:, 0:513],
                                 in1=tmp[:, 0:513])
            u_tiles[(h, yc)] = u_t
        # ragged y = 512 row
        ps_rr = pf.tile([128, 520], F32, tag="pf", name=f"psrr{b}_{h}")
        ps_ri = pf.tile([128, 520], F32, tag="pf", name=f"psri{b}_{h}")
        nc.tensor.matmul(ps_rr[0:1, 0:512], kstk[h][:, 512:513],
                         qstk[h][:, 0:512], start=True, stop=True)
        nc.tensor.matmul(ps_rr[0:1, 512:514], kstk[h][:, 512:513],
                         qstk[h][:, 512:514], start=True, stop=True)
        nc.tensor.matmul(ps_ri[0:1, 0:512], kswp[h][:, 512:513],
                         qstk[h][:, 0:512], start=True, stop=True)
        nc.tensor.matmul(ps_ri[0:1, 512:514], kswp[h][:, 512:513],
                         qstk[h][:, 512:514], start=True, stop=True)
        ra = sm.tile([1, X], F32, tag="ra", bufs=2, name=f"raga{b}_{h}")
        rb = sm.tile([1, X], F32, tag="rb", bufs=2, name=f"ragb{b}_{h}")
        nc.vector.tensor_scalar_mul(out=ra[0:1, :], in0=ps_rr[0:1, 0:513],
                                    scalar1=1.0)
        nc.vector.tensor_scalar_mul(out=rb[0:1, :], in0=ps_ri[0:1, 0:513],
                                    scalar1=1.0)
        nc.sync.dma_start(out=rag_s[i:i + 1, :], in_=ra[0:1, :])
        nc.sync.dma_start(out=rag_i[i:i + 1, :], in_=rb[0:1, :])

    # ---- ragged magnitude (small) -----------------------------------
    nc.gpsimd.tensor_mul(out=rag_s[:, :], in0=rag_s[:, :], in1=rag_s[:, :])
    nc.gpsimd.tensor_mul(out=rag_i[:, :], in0=rag_i[:, :], in1=rag_i[:, :])
    nc.gpsimd.tensor_add(out=rag_s[:, :], in0=rag_s[:, :], in1=rag_i[:, :])

    # ---- sqrt segment (sqrt table set) ------------------------------
    for h in heads:
        for yc in range(NYC):
            u_t = u_tiles[(h, yc)]
            nc.scalar.sqrt(out=u_t[:, 0:513], in_=u_t[:, 0:513])
    nc.scalar.sqrt(out=rag_s[:, :], in_=rag_s[:, :])

    # ---- exp segment (exp table set); fp8 out, bias -ln4 ------------
    exp_tiles = {}
    for h in heads:
        for yc in range(NYC):
            u_t = u_tiles[(h, yc)]
            e_view = u_t.bitcast(F8)
            nc.scalar.activation(out=e_view[:, 0:X], in_=u_t[:, 0:513],
                                 func=AF.Exp, bias=ebias[:, 0:1])
            exp_tiles[(h, yc)] = e_view
    rag_p = wv.tile([4, X], F8, tag="ragp", bufs=2, name=f"ragp{b}_{w}")
    nc.scalar.activation(out=rag_p[:, :], in_=rag_s[:, :], func=AF.Exp,
                         bias=ebias[0:4, 0:1])
    exp_rag = {}
    for i, h in enumerate(heads):
        er = wv.tile([1, X], F8, tag="exprag", bufs=6, name=f"er{b}_{h}")
        exp_rag[h] = er
        nc.sync.dma_start(out=er[0:1, :], in_=rag_p[i:i + 1, :])

    # ---- AV (mixed fp8 lhsT x bf16 rhs) + colsum normalization ------
    for h in heads:
        for xc in range(NXC):
            xcs = slice(xc * 128, (xc + 1) * 128)
            ps_av = ph.tile([128, 512], F32, tag="ph", name=f"psav{b}_{h}_{xc}")
            for yc in range(NYC):
                nc.tensor.matmul(ps_av[:, 0:129], exp_tiles[(h, yc)][:, xcs],
                                 vf_av[yc][:, h, 0:129],
                                 start=(yc == 0), stop=False)
            nc.tensor.matmul(ps_av[:, 0:129], exp_rag[h][0:1, xcs],
                             v512[0:1, h, 0:129], start=False, stop=True)
            rc = sm.tile([128, 1], F32, tag="rc", bufs=4, name=f"rc{b}_{h}_{xc}")
            nc.vector.reciprocal(out=rc[:, :], in_=ps_av[:, 128:129])
            nc.vector.tensor_scalar(
                out=qkv8[:, xc, :, h, :],
                in0=ps_av[:, 0:128].rearrange("p (t e) -> p t e", t=2),
                scalar1=rc[:, 0:1], scalar2=SQKV,
                op0=OPS.mult, op1=OPS.mult)
        # ragged x = 512 row
        ps_a1 = ph.tile([128, 512], F32, tag="ph", name=f"psa1{b}_{h}")
        for yc in range(NYC):
            nc.tensor.matmul(ps_a1[0:1, 0:129], exp_tiles[(h, yc)][:, 512:513],
                             vf_av[yc][:, h, 0:129],
                             start=(yc == 0), stop=False)
        nc.tensor.matmul(ps_a1[0:1, 0:129], exp_rag[h][0:1, 512:513],
                         v512[0:1, h, 0:129], start=False, stop=True)
        rc1 = sm.tile([1, 1], F32, tag="rc1", bufs=2, name=f"rc1{b}_{h}")
        nc.vector.reciprocal(out=rc1[0:1, :], in_=ps_a1[0:1, 128:129])
        nc.vector.tensor_scalar(
            out=qkv512[0:1, :, h, :],
            in0=ps_a1[0:1, 0:128].rearrange("p (t e) -> p t e", t=2),
            scalar1=rc1[0:1, 0:1], scalar2=SQKV,
            op0=OPS.mult, op1=OPS.mult)


_BUILT = None
_CONSTS = None


def _get_built():
    global _BUILT, _CONSTS
    if _BUILT is None:
        _BUILT = build_module()
        _CONSTS = _consts()
    return _BUILT, _CONSTS


def kernel(q, k, v, mask, tau, delta):
    global LAST_RESULT
    nc, (fre, fim, fre8, fim8, gre8, gim8) = _get_built()
    q = np.ascontiguousarray(np.asarray(q, dtype=np.float32)).astype(NPF8)
    k = np.ascontiguousarray(np.asarray(k, dtype=np.float32)).astype(NPF8)
    v = np.ascontiguousarray(np.asarray(v, dtype=np.float32)).astype(NPBF)
    # iFFT psum carries SF_g*SQKV = 16*64 = 1024x scale; fold into tau
    tau = np.ascontiguousarray(np.asarray(tau, dtype=np.float32)) / (SG * SQKV)
    delta = np.ascontiguousarray(np.asarray(delta, dtype=np.float32))

    in_maps = []
    for i in range(NCORES):
        sl = slice(i * BL, (i + 1) * BL)
        in_maps.append({
            "qd": np.ascontiguousarray(q[sl]),
            "kd": np.ascontiguousarray(k[sl]),
            "vd": np.ascontiguousarray(v[sl]),
            "taud": np.ascontiguousarray(tau[sl]),
            "deltad": np.ascontiguousarray(delta[sl]),
            "fred": fre, "fimd": fim, "fre8d": fre8, "fim8d": fim8,
            "gre8d": gre8, "gim8d": gim8,
        })
    res = run_bass_kernel_spmd(nc, in_maps, core_ids=list(range(NCORES)))
    LAST_RESULT = res
    out = np.concatenate([res.results[i]["outd"] for i in range(NCORES)], axis=0)
    return out.astype(np.float32)
